# revision 33
# baseline (speedup 1.0000x reference)
"""Dense dot-product attention with key-length masking on 8 Trainium2 cores.

Problem: q,k,v [16, 2048, 128] fp32, valid_lens [16,1] int32.
  out = softmax(mask(q@k.T/sqrt(d))) @ v   (masked keys -> -1e6 before softmax)

Device algorithm (per work unit = one batch x one 1024-query slice):
- Host pre-transposes q,k to [d, seq] (fp16); the device never transposes.
- S^T tiles (keys on partitions): the key mask is a per-partition
  scale/bias folded into the exp() activation:
     E = exp(S_raw * scale_k + bias_k), scale_k = m_k/sqrt(d), bias_k = -30*(1-m_k)
  For valid_len==0 the host sets scale=bias=0 -> E=1 -> uniform softmax,
  matching the reference's where(mask, w, NEG) semantics exactly.
- O^T accumulates over key tiles with V (fp16) stationary, E (fp16) moving.
- Softmax denominators: DVE pairwise fp16 tree over E tiles (2x mode);
  host finishes the 128-partition sum and does the divide + transpose.
- HAM warm-up: dummy bf16 matmuls run while the input DMAs stream, so
  the PE clock-gate is at 8/8 when real compute starts.

Work distribution (valid_lens-aware, single SPMD program):
- 32 units (16 batches x 2 query halves), work(unit) = ceil(L/128) key
  tiles (16 for L==0: uniform softmax must cover every key).
- Units sorted by work, slot j of every core takes one of ranks
  [8j, 8j+8); the per-slot key-tile trip count baked at build time is
  rank 8j's work, so the SPMD program wastes almost nothing and cores
  are balanced to ~1/8 of total work each.
"""

import math
import sys
import types

import numpy as np

import concourse.bass as bass
import concourse.mybir as mybir
import concourse.tile as tile
from concourse import bacc
from concourse.bass_utils import run_bass_kernel_spmd

B, Q, K, D = 16, 2048, 2048, 128
NCORES = 8
QCH = 1024         # queries per work unit
UNITS = B * (Q // QCH)
NSLOT = UNITS // NCORES
MM_N = 512         # moving-operand free dim per matmul
KT = K // 128      # max key tiles
SCALE = 1.0 / math.sqrt(D)
NEG_BIAS = -30.0   # exp(-30) ~ 1e-13: invisible next to real softmax terms
WARMUP_MMS = 10    # dummy matmuls to lift the PE HAM clock-gate

F32 = mybir.dt.float32
F16 = mybir.dt.float16
BF16 = mybir.dt.bfloat16


def _install_hook_stub():
    """bass_utils' axon trace path imports antenv.axon_hooks, which is not
    shipped in this container.  Provide a no-op stub so an ambient
    BASS_TRACE=1 doesn't crash; test harnesses may overwrite the hook."""
    if "antenv.axon_hooks" in sys.modules:
        return
    mod = types.ModuleType("antenv.axon_hooks")
    _hook = [None]
    mod.set_axon_ntff_profile_hook = lambda h: _hook.__setitem__(0, h)
    mod.get_axon_ntff_profile_hook = lambda: _hook[0]
    sys.modules["antenv.axon_hooks"] = mod


_install_hook_stub()

_build_cache = {}
last_result = None  # BassKernelResults of the most recent run (for harnesses)


def _build(trips, nfull):
    """One SPMD program: slot j processes trips[j] key tiles of one unit.
    The first nfull[j] key tiles are fully valid on every core, so their
    exp() uses immediate scale/bias (no per-partition operand fetch)."""
    nc = bacc.Bacc(num_devices=NCORES)

    qT = nc.declare_dram_parameter("qT", [NSLOT, D, QCH], F16, isOutput=False)
    kT = nc.declare_dram_parameter("kT", [NSLOT, D, K], F16, isOutput=False)
    v = nc.declare_dram_parameter("v", [NSLOT, K, D], F16, isOutput=False)
    sc = nc.declare_dram_parameter("sc", [NSLOT, 128, KT], F32, isOutput=False)
    bi = nc.declare_dram_parameter("bi", [NSLOT, 128, KT], F32, isOutput=False)
    oT = nc.declare_dram_parameter("oT", [NSLOT, D, QCH], F32, isOutput=True)
    # per-unit partial softmax denominators: host finishes the 128-way sum
    esum = nc.declare_dram_parameter("esum", [NSLOT, 128, QCH], F16, isOutput=True)

    with tile.TileContext(nc) as tc:
        with (
            tc.tile_pool(name="consts", bufs=1) as consts,
            tc.tile_pool(name="inputs", bufs=2) as inpool,
            tc.tile_pool(name="epool", bufs=max(trips) + 8) as epool,
            tc.tile_pool(name="treep", bufs=3) as treepool,
            tc.tile_pool(name="osb", bufs=2) as opool,
            tc.tile_pool(name="sps", bufs=3, space="PSUM") as pspool,
            tc.tile_pool(name="oacc", bufs=1, space="PSUM") as psacc,
        ):
            # --- HAM warm-up: dummy bf16 matmuls while input DMAs stream ---
            wsrc = consts.tile([128, MM_N], BF16)
            nc.vector.memset(wsrc[:], 1.0)
            for w in range(WARMUP_MMS):
                if w % 2 == 0:
                    wps = pspool.tile([128, QCH], F32, tag="s")
                nc.tensor.matmul(
                    wps[:, (w % 2) * MM_N : (w % 2) * MM_N + MM_N],
                    wsrc[:, :128],
                    wsrc[:],
                    start=True,
                    stop=True,
                    skip_group_check=True,
                )

            for s in range(NSLOT):
                t = trips[s]
                qT_sb = inpool.tile([128, QCH], F16, tag="qT")
                kT_sb = inpool.tile([128, t * 128], F16, tag="kT")
                v_sb = inpool.tile([128, t, D], F16, tag="v")
                sc_sb = inpool.tile([128, KT], F32, tag="sc")
                bi_sb = inpool.tile([128, KT], F32, tag="bi")
                nc.sync.dma_start(out=sc_sb[:], in_=sc[s])
                nc.sync.dma_start(out=bi_sb[:], in_=bi[s])
                # split big input DMAs so compute starts on the first pieces;
                # slot 0 gates pipeline start, so spread it over many queues
                pk = 4  # key tiles per DMA piece
                nq = 1  # qT pieces
                for j in range(nq):
                    nc.sync.dma_start(
                        out=qT_sb[:, bass.ts(j, QCH // nq)],
                        in_=qT[s][:, bass.ts(j, QCH // nq)],
                    )
                v_dram = v[s].rearrange("(i p) d -> p i d", p=128)
                ntp = (t + pk - 1) // pk
                for j in range(ntp):
                    klo, khi = j * pk * 128, min(t * 128, (j + 1) * pk * 128)
                    nc.sync.dma_start(out=kT_sb[:, klo:khi], in_=kT[s][:, klo:khi])
                    tlo, thi = j * pk, min(t, (j + 1) * pk)
                    nc.sync.dma_start(
                        out=v_sb[:, tlo:thi, :], in_=v_dram[:, tlo:thi, :]
                    )

                etiles = []
                o_ps = psacc.tile([128, QCH], F32, tag="o")
                for i in range(t):
                    s_ps = pspool.tile([128, QCH], F32, tag="s")
                    for h in range(QCH // MM_N):
                        nc.tensor.matmul(
                            s_ps[:, bass.ts(h, MM_N)],
                            kT_sb[:, bass.ts(i, 128)],
                            qT_sb[:, bass.ts(h, MM_N)],
                            start=True,
                            stop=True,
                        )
                    e_sb = epool.tile([128, QCH], F16, tag="e")
                    etiles.append(e_sb)
                    if i < nfull[s]:
                        nc.scalar.activation(
                            e_sb[:],
                            s_ps[:],
                            mybir.ActivationFunctionType.Exp,
                            scale=float(SCALE),
                        )
                    else:
                        nc.scalar.activation(
                            e_sb[:],
                            s_ps[:],
                            mybir.ActivationFunctionType.Exp,
                            bias=bi_sb[:, i : i + 1],
                            scale=sc_sb[:, i : i + 1],
                        )
                    for h in range(QCH // MM_N):
                        nc.tensor.matmul(
                            o_ps[:, bass.ts(h, MM_N)],
                            v_sb[:, i, :],
                            e_sb[:, bass.ts(h, MM_N)],
                            start=(i == 0),
                            stop=(i == t - 1),
                        )

                # denominator: DVE pairwise fp16 tree (2x mode) down to one
                # [128, QCH] survivor; host finishes the partition sum
                cur = [e[:] for e in etiles]
                if len(cur) > 1:
                    tr = treepool.tile([128, (t + 1) // 2, QCH], F16, tag="tr")
                    nxt = []
                    for j in range(len(cur) // 2):
                        nc.vector.tensor_add(tr[:, j, :], cur[2 * j], cur[2 * j + 1])
                        nxt.append(tr[:, j, :])
                    if len(cur) % 2:
                        nxt.append(cur[-1])
                    cur = nxt
                    while len(cur) > 1:
                        nxt = []
                        for j in range(len(cur) // 2):
                            nc.vector.tensor_add(cur[2 * j], cur[2 * j], cur[2 * j + 1])
                            nxt.append(cur[2 * j])
                        if len(cur) % 2:
                            nxt.append(cur[-1])
                        cur = nxt
                nc.sync.dma_start(out=esum[s], in_=cur[0])

                o_sb = opool.tile([128, QCH], F32, tag="osb")
                for h in range(2):
                    hs = bass.ts(h, QCH // 2)
                    nc.vector.tensor_copy(o_sb[:, hs], o_ps[:, hs])
                    nc.sync.dma_start(out=oT[s][:, hs], in_=o_sb[:, hs])

    nc.compile()
    return nc


def kernel(q, k, v, valid_lens):
    q = np.ascontiguousarray(q, dtype=np.float32)
    k = np.ascontiguousarray(k, dtype=np.float32)
    v = np.ascontiguousarray(v, dtype=np.float32)
    L = np.asarray(valid_lens).reshape(-1).astype(np.int64)

    # per-batch key-tile need; L==0 must cover all keys (uniform softmax)
    need = np.where(L == 0, KT, np.minimum(KT, (L + 127) // 128)).astype(np.int64)

    # units: (batch, q-half) ranked by work ascending into NSLOT groups of
    # 8; group order in the schedule puts a small group first (its input
    # load gates compute start) and the smallest last (its denominator
    # tree is the exposed tail)
    units = [(int(need[b]), b, h) for b in range(B) for h in range(Q // QCH)]
    units.sort(key=lambda u: u[0])
    group_order = [1, NSLOT - 1] + list(range(NSLOT - 2, 1, -1)) + [0]
    trips = tuple(units[NCORES * g + NCORES - 1][0] for g in group_order)
    # leading key tiles fully valid on every core of the slot (and not the
    # L==0 uniform case) can use immediate scale/bias in exp()
    nfull = []
    for g in group_order:
        group = units[NCORES * g : NCORES * (g + 1)]
        ls = [int(L[b]) for _, b, _ in group]
        nfull.append(0 if min(ls) == 0 else min(l // 128 for l in ls))
    nfull = tuple(nfull)

    key = (trips, nfull)
    if key not in _build_cache:
        _build_cache[key] = _build(trips, nfull)
    nc = _build_cache[key]

    qh = q.astype(np.float16)
    kh = k.astype(np.float16)
    vh = v.astype(np.float16)

    kidx = np.arange(K)
    sc_all = np.empty((B, 128, KT), np.float32)
    bi_all = np.empty((B, 128, KT), np.float32)
    for b in range(B):
        lb = int(L[b])
        if lb == 0:
            scv = np.zeros(K, np.float32)
            biv = np.zeros(K, np.float32)
        else:
            m = (kidx < lb).astype(np.float32)
            scv = m * np.float32(SCALE)
            biv = (1.0 - m) * np.float32(NEG_BIAS)
        sc_all[b] = scv.reshape(KT, 128).T
        bi_all[b] = biv.reshape(KT, 128).T

    in_maps = []
    core_units = []  # [core][slot] -> (b, h)
    for c in range(NCORES):
        slots = [units[NCORES * g + c] for g in group_order]
        core_units.append([(b, h) for _, b, h in slots])
        qT = np.stack(
            [np.ascontiguousarray(qh[b, h * QCH : (h + 1) * QCH].T) for _, b, h in slots]
        )
        kTt = np.stack([np.ascontiguousarray(kh[b].T) for _, b, _ in slots])
        vv = np.stack([vh[b] for _, b, _ in slots])
        in_maps.append(
            {
                "qT": qT,
                "kT": kTt,
                "v": vv,
                "sc": np.ascontiguousarray(np.stack([sc_all[b] for _, b, _ in slots])),
                "bi": np.ascontiguousarray(np.stack([bi_all[b] for _, b, _ in slots])),
            }
        )

    res = run_bass_kernel_spmd(nc, in_maps, list(range(NCORES)))
    global last_result
    last_result = res

    out = np.empty((B, Q, D), np.float32)
    for c in range(NCORES):
        r = res.results[c]
        for s in range(NSLOT):
            b, h = core_units[c][s]
            sums = r["esum"][s].astype(np.float32).sum(axis=0)  # [QCH]
            out[b, h * QCH : (h + 1) * QCH] = (r["oT"][s].astype(np.float32) / sums[None, :]).T
    return out
